# revision 52
# baseline (speedup 1.0000x reference)
"""Trainium2 Bass kernel for per-node multi-head attention.

Computation (per node n, fully independent across nodes):
    Q = h @ Wq.T  viewed (nh, hd)        [row-major reshape]
    K = h @ Wk.T  viewed (hd, nh)
    V = h @ Wv.T  viewed (hd, nh)
    comp[hh, g] = sum_d Q[hh, d] K[d, g] / 128
    scores = softmax(comp, axis=-1)
    out[l, d]  = sum_g scores[l, g] V[d, g]
    final = flat(out.T) @ Wfc.T

Sharding: data-parallel over the node dim N across 8 NeuronCores; weights
replicated; no collectives.

Per-core mapping:
  - TensorE: transpose h blocks (128x128), the 3 projections + final FC as
    bf16 matmuls (1 cycle/row), transpose of the attention output.
  - VectorE: the batched per-node einsum products as broadcast-AP elementwise
    bf16 multiplies (2x DVE mode) + the first levels of the bf16 pairwise-add
    reduction trees, plus softmax normalization.
  - GpSimd (Pool engine): the tail levels of the reduction trees (it cannot
    touch PSUM, but SBUF-to-SBUF bf16 adds are fine), taking adds off the
    critical VectorE path. comp-tree tails gate softmax so they are kept
    short; out-tree tails gate only the PE tail, so whole out trees go to
    GpSimd and their emission is deferred one tile so the in-order Pool queue
    serves comp tails first.
  - ScalarE: exp, PSUM->SBUF copies (with dtype casts).
  - Wk's rows are permuted during on-device weight prep so the K projection
    lands g-major (f' = g*64 + d), which makes the comp product APs unit-
    stride in the innermost dim.

Scheduling: engines execute their queues in order, so emission order is the
schedule. The loop is software-pipelined two tiles deep (projections of tile
i+1 and the outT+FC of tile i-1 are emitted inside tile i's attention phase),
h DMAs are prefetched two tiles ahead of the out-DMAs they share the SP queue
with, and the per-pass engine tables (COMP_TREES/OUT_TREES/PLAN) were tuned
against the TimelineSim cost model.
"""

import numpy as np

N_FULL = 65536
H = 1024
NCORES = 8
NPC = N_FULL // NCORES  # rows per core
NH = 16                 # heads
HD = 64                 # head dim
KT = H // 128           # k tiles per contraction (8)

# Engine assignment for reduction-tree levels ('D' = VectorE, 'P' = GpSimd),
# one tuple per pass. Early passes lean on GpSimd (their latency is hidden by
# later passes); the last pass stays on VectorE so the end-of-tile tail is
# short. comp tree: 6 levels (seg width 64 -> 1); out tree: 4 levels (16 -> 1).
COMP_TREES = (
    ("D", "D", "D", "P", "P", "P"),
    ("D", "D", "D", "P", "P", "P"),
    ("D", "D", "D", "D", "D", "D"),
    ("D", "D", "D", "D", "D", "D"),
)
OUT_TREES = (("P", "P", "P", "P"),) * 2 + (("D", "P", "P", "P"),) * 2

_BUILD_CACHE = {}


# Emission plan for one tile's attention phase: ("p1",q) product,
# ("ct",q) comp tree, ("sm",q) softmax, ("out",q) p2+out tree.
PLAN = (
    ("p1", 0), ("ct", 0), ("p1", 1), ("ct", 1), ("p1", 2), ("ct", 2),
    ("p1", 3), ("ct", 3),
    ("sm", 0), ("sm", 1), ("sm", 2), ("sm", 3),
    ("out", 0), ("out", 1), ("out", 2), ("out", 3),
)


def _build(n_rows, comp_trees=COMP_TREES, out_trees=OUT_TREES, plan=PLAN):
    key = (n_rows, tuple(comp_trees), tuple(out_trees), tuple(plan))
    if key in _BUILD_CACHE:
        return _BUILD_CACHE[key]

    import concourse.bass as bass
    import concourse.mybir as mybir
    import concourse.tile as tile
    from concourse import bacc
    from concourse.masks import make_identity

    f32 = mybir.dt.float32
    f32r = mybir.dt.float32r
    bf16 = mybir.dt.bfloat16
    MULT = mybir.AluOpType.mult
    ADD = mybir.AluOpType.add
    AXX = mybir.AxisListType.X

    nc = bacc.Bacc("TRN2", target_bir_lowering=False, debug=False)

    h_d = nc.dram_tensor("h", [n_rows, H], f32, kind="ExternalInput").ap()
    w_d = {
        name: nc.dram_tensor(name, [H, H], f32, kind="ExternalInput").ap()
        for name in ("wq", "wk", "wv", "wfc")
    }
    out_d = nc.dram_tensor("out", [n_rows, H], f32, kind="ExternalOutput").ap()

    ntiles = n_rows // 128

    def ap(base, offset_elems, dims):
        """Manual AP: dims are [step, count] FREE dims; partition from base."""
        b = base if isinstance(base, bass.AP) else base[...]
        return bass.AP(
            tensor=b.tensor,
            offset=b.offset + offset_elems,
            ap=[list(b.ap[0])] + [list(d) for d in dims],
        )

    def eng(which):
        return nc.vector if which == "D" else nc.gpsimd

    def tree(src, segs, width, dst_f32, assign, scratch, defer_p=False):
        """Pairwise-add reduce: src holds segs segments of `width` bf16 elems
        (contiguous); reduce innermost to 1, writing f32 into dst_f32 (AP with
        `segs` contiguous elems). Intermediate levels go to `scratch` (bf16
        tile, >= segs*width/2 elems). assign[i] picks the engine per level.
        With defer_p=True, emission stops at the first GpSimd level and a
        continuation emitting the rest is returned (so those Pool-queue
        entries can be enqueued after higher-priority Pool work)."""
        state = [0, src, width, 0, 0]

        def emit(rest_only):
            off_in, buf_in, w, lvl, off_out = state
            while w > 2:
                if defer_p and not rest_only and assign[lvl] == "P":
                    return False
                half = w // 2
                e = eng(assign[lvl])
                e.tensor_tensor(
                    ap(scratch, off_out, [[half, segs], [1, half]]),
                    ap(buf_in, off_in, [[w, segs], [1, half]]),
                    ap(buf_in, off_in + half, [[w, segs], [1, half]]),
                    ADD,
                )
                off_in, buf_in = off_out, scratch
                off_out += segs * half
                w, lvl = half, lvl + 1
                state[:] = [off_in, buf_in, w, lvl, off_out]
            if defer_p and not rest_only and assign[lvl] == "P":
                return False
            e = eng(assign[lvl])
            e.tensor_tensor(
                dst_f32,
                ap(buf_in, off_in, [[2, segs]]),
                ap(buf_in, off_in + 1, [[2, segs]]),
                ADD,
            )
            state[3] = -1  # done
            return True

        if emit(rest_only=False):
            return None
        return lambda: emit(rest_only=True)

    with tile.TileContext(nc) as tc:
        with tc.tile_pool(name="const", bufs=1) as const_pool:
            ident = const_pool.tile([128, 128], f32)
            make_identity(nc, ident)
            identb = const_pool.tile([128, 128], bf16)
            make_identity(nc, identb)

            # Transposed weights, SBUF-resident for the whole kernel.
            # wt[p, kt, f] = W[f, kt*128 + p]   (for wk: f is permuted g-major)
            wts = {}
            with tc.tile_pool(name="wprep", bufs=2) as wnat_pool, \
                 tc.tile_pool(name="wtps", bufs=4, space="PSUM") as wt_psum:
                for name in ("wq", "wk", "wv", "wfc"):
                    wt = const_pool.tile([128, KT, H], bf16, tag=f"wt_{name}")
                    wts[name] = wt
                    wnat = wnat_pool.tile([128, KT, H], f32, tag="wnat")
                    nc.sync.dma_start(
                        out=wnat,
                        in_=w_d[name].rearrange("(ft p) c -> p ft c", p=128),
                    )
                    for ft in range(KT):
                        for kt in range(KT):
                            ps = wt_psum.tile([128, 128], f32, tag="wt_ps")
                            nc.tensor.transpose(
                                ps[:, :], wnat[:, ft, kt * 128:(kt + 1) * 128],
                                ident[:, :],
                            )
                            ceng = (nc.scalar, nc.vector)[kt % 2]  # gpsimd can't read PSUM
                            if name == "wk":
                                # permute output features to g-major:
                                # f = 16*dl + g + 128*ft  ->  f' = 64*g + 8*ft + dl
                                src = ap(ps, 0, [[16, 8], [1, 16]])       # (dl, g)
                                dst = ap(wt, kt * H + 8 * ft,
                                         [[1, 8], [64, 16]])              # (dl, g)
                                if ceng is nc.scalar:
                                    ceng.copy(out=dst, in_=src)
                                else:
                                    ceng.tensor_copy(out=dst, in_=src)
                            else:
                                dst = wt[:, kt, ft * 128:(ft + 1) * 128]
                                if ceng is nc.scalar:
                                    ceng.copy(out=dst, in_=ps[:, :])
                                else:
                                    ceng.tensor_copy(out=dst, in_=ps[:, :])

            with tc.tile_pool(name="io", bufs=3) as io_pool, \
                 tc.tile_pool(name="acts", bufs=3) as act_pool, \
                 tc.tile_pool(name="prod", bufs=5) as prod_pool, \
                 tc.tile_pool(name="trp", bufs=4) as tr_pool, \
                 tc.tile_pool(name="small", bufs=2) as small_pool, \
                 tc.tile_pool(name="tps", bufs=2, space="PSUM") as t_psum, \
                 tc.tile_pool(name="mmps", bufs=4, space="PSUM") as mm_psum:

                def load_h(it):
                    r0 = it * 128
                    h_sb = io_pool.tile([128, H], f32, tag="h")
                    nc.sync.dma_start(out=h_sb, in_=h_d[r0:r0 + 128, :])
                    return h_sb

                def load_and_project(it, h_sb):
                    """Transpose the h tile, run the 3 projections."""
                    # hT[p, c, j] = h[r0 + j, c*128 + p] via PE transposes;
                    # fine-grained so the first proj matmuls start after the
                    # first 128-block lands.
                    hT = act_pool.tile([128, KT, 128], bf16, tag="hT")
                    for c in range(KT):
                        ps = t_psum.tile([128, 128], f32, tag="tp")
                        nc.tensor.transpose(
                            ps[:, :], h_sb[:, c * 128:(c + 1) * 128], ident[:, :]
                        )
                        nc.scalar.copy(out=hT[:, c, :], in_=ps[:, :])

                    # Projections -> bf16 activations.
                    # qb: (hh, d) row-major;  kb: (g, d) [via permuted wk];
                    # vb: (d, g) row-major.
                    projs = {}
                    for name, pname in (("wq", "qb"), ("wk", "kb"), ("wv", "vb")):
                        dst = act_pool.tile([128, H], bf16, tag=pname)
                        projs[pname] = dst
                        for half in range(2):
                            ps = mm_psum.tile([128, 512], f32, tag="mm")
                            for kt in range(KT):
                                nc.tensor.matmul(
                                    ps[:, :],
                                    hT[:, kt, :],
                                    wts[name][:, kt, half * 512:(half + 1) * 512],
                                    start=(kt == 0),
                                    stop=(kt == KT - 1),
                                )
                            nc.scalar.copy(
                                out=dst[:, half * 512:(half + 1) * 512], in_=ps[:, :]
                            )
                    return projs["qb"], projs["kb"], projs["vb"]

                def attention_parts(qb, kb, vb):
                    """Explicitly list-scheduled emission. Engines execute
                    their queues in order, so emission order is chosen so no
                    queue head ever waits while ready work sits behind it:
                    all products first (Pool trees launched as soon as their
                    product lands), then VectorE trees, then softmax + out
                    passes ordered by comp readiness (D-tree quarters first)."""
                    comp = small_pool.tile([128, NH, NH], f32, tag="comp")
                    e = small_pool.tile([128, NH, NH], f32, tag="e")
                    s = small_pool.tile([128, NH], f32, tag="s")
                    r = small_pool.tile([128, NH], f32, tag="r")
                    scores = small_pool.tile([128, NH, NH], bf16, tag="sc")
                    OUT = act_pool.tile([128, H], bf16, tag="out")

                    def p1_mult(qq):
                        p1 = prod_pool.tile([128, 4, NH, HD], bf16, tag="prod")
                        in0 = ap(qb, qq * 4 * HD, [[HD, 4], [0, NH], [1, HD]])
                        in1 = ap(kb, 0, [[0, 4], [HD, NH], [1, HD]])
                        nc.vector.tensor_tensor(p1[...], in0, in1, MULT)
                        return p1

                    def comp_tree(qq, p1):
                        tr = tr_pool.tile([128, 4096], bf16, tag="tr")
                        tree(p1, 64, HD, comp[:, qq * 4:(qq + 1) * 4, :],
                             comp_trees[qq], tr)

                    def softmax(qq):
                        if qq < 0:  # merged: all 16 heads in one shot
                            sl = slice(0, NH)
                            nc.scalar.activation(
                                e[:, sl, :], comp[:, sl, :],
                                mybir.ActivationFunctionType.Exp,
                                scale=1.0 / 128.0,
                            )
                            nc.vector.tensor_reduce(
                                s[:, sl], e[:, sl, :], AXX, ADD)
                            nc.vector.reciprocal(r[:, sl], s[:, sl])
                            nc.vector.tensor_tensor(
                                scores[:, sl, :], e[:, sl, :],
                                ap(r, 0, [[1, NH], [0, NH]]), MULT
                            )
                            return
                        sl = slice(qq * 4, (qq + 1) * 4)
                        nc.scalar.activation(
                            e[:, sl, :], comp[:, sl, :],
                            mybir.ActivationFunctionType.Exp, scale=1.0 / 128.0,
                        )
                        nc.vector.tensor_reduce(s[:, sl], e[:, sl, :], AXX, ADD)
                        nc.vector.reciprocal(r[:, sl], s[:, sl])
                        nc.vector.tensor_tensor(
                            scores[:, sl, :], e[:, sl, :],
                            ap(r, qq * 4, [[1, 4], [0, NH]]), MULT
                        )

                    deferred = []

                    def out_pass(qq):
                        # out[l, d] = sum_g scores[l, g]*vb[d*16+g]; the l
                        # quarter == this score quarter. OUT flat = 16*d+l.
                        p2 = prod_pool.tile([128, 4, HD, NH], bf16, tag="prod")
                        in0 = ap(scores, qq * 4 * NH,
                                 [[NH, 4], [0, HD], [1, NH]])
                        in1 = ap(vb, 0, [[0, 4], [NH, HD], [1, NH]])
                        nc.vector.tensor_tensor(p2[...], in0, in1, MULT)
                        tr = tr_pool.tile([128, 4096], bf16, tag="tr")
                        cont = tree(p2, 256, NH,
                                    ap(OUT, qq * 4, [[1, 4], [NH, HD]]),
                                    out_trees[qq], tr, defer_p=True)
                        if cont is not None:
                            deferred.append(cont)

                    p1s = {}

                    def run(which):
                        for op, qq in plan:
                            if op == "out" and which == "out":
                                out_pass(qq)
                            elif op != "out" and which == "comp":
                                if op == "p1":
                                    p1s[qq] = p1_mult(qq)
                                elif op == "ct":
                                    comp_tree(qq, p1s[qq])
                                elif op == "sm":
                                    softmax(qq)
                    return run, OUT, deferred

                def transpose_fc(it, OUT):
                    """outT[p, c, j] = OUT[j, c*128 + p]; final = outT @ WfcT."""
                    r0 = it * 128
                    outT = act_pool.tile([128, KT, 128], bf16, tag="outT")
                    for c in range(KT):
                        ps = t_psum.tile([128, 128], bf16, tag="tpb")
                        nc.tensor.transpose(
                            ps[:, :], OUT[:, c * 128:(c + 1) * 128], identb[:, :]
                        )
                        nc.scalar.copy(out=outT[:, c, :], in_=ps[:, :])

                    final = io_pool.tile([128, H], f32, tag="final")
                    for half in range(2):
                        ps = mm_psum.tile([128, 512], f32, tag="mm")
                        for kt in range(KT):
                            nc.tensor.matmul(
                                ps[:, :],
                                outT[:, kt, :],
                                wts["wfc"][:, kt, half * 512:(half + 1) * 512],
                                start=(kt == 0),
                                stop=(kt == KT - 1),
                            )
                        nc.scalar.copy(
                            out=final[:, half * 512:(half + 1) * 512], in_=ps[:, :]
                        )
                    nc.sync.dma_start(out=out_d[r0:r0 + 128, :], in_=final)

                # Software pipeline, two tiles deep. Emission order per
                # iteration: comp-phase(it) | load+proj(it+1) | outT+FC(it-1)
                # | out-phase(it). This keeps the next tile's projection
                # chain (PE+Act) ahead of this tile's late exp/copies in the
                # in-order Act queue, and ahead of the FC in the PE queue,
                # so qb/kb/vb(it+1) are ready before the boundary.
                h_bufs = {0: load_h(0)}
                if ntiles > 1:
                    h_bufs[1] = load_h(1)
                qkv = load_and_project(0, h_bufs.pop(0))
                pending = None  # (it, OUT, deferred) awaiting tails + FC
                for it in range(ntiles):
                    if it + 2 < ntiles:
                        h_bufs[it + 2] = load_h(it + 2)
                    run, OUT, deferred = attention_parts(*qkv)
                    run("comp")
                    if it + 1 < ntiles:
                        qkv = load_and_project(it + 1, h_bufs.pop(it + 1))
                    if pending is not None:
                        p_it, p_out, p_def = pending
                        for cont in p_def:
                            cont()
                        transpose_fc(p_it, p_out)
                    run("out")
                    pending = (it, OUT, deferred)
                p_it, p_out, p_def = pending
                for cont in p_def:
                    cont()
                transpose_fc(p_it, p_out)

    nc.compile()
    _BUILD_CACHE[key] = nc
    return nc


def kernel(h, Wq, Wk, Wv, Wfc):
    from concourse import bass_utils

    h = np.ascontiguousarray(np.asarray(h, dtype=np.float32))
    ws = {
        "wq": np.ascontiguousarray(np.asarray(Wq, dtype=np.float32)),
        "wk": np.ascontiguousarray(np.asarray(Wk, dtype=np.float32)),
        "wv": np.ascontiguousarray(np.asarray(Wv, dtype=np.float32)),
        "wfc": np.ascontiguousarray(np.asarray(Wfc, dtype=np.float32)),
    }
    nc = _build(NPC)
    in_maps = [
        {"h": h[i * NPC:(i + 1) * NPC], **ws} for i in range(NCORES)
    ]
    res = bass_utils.run_bass_kernel_spmd(nc, in_maps, core_ids=list(range(NCORES)))
    return np.concatenate(
        [res.results[i]["out"] for i in range(NCORES)], axis=0
    ).astype(np.float32)
